# revision 54
# baseline (speedup 1.0000x reference)
"""DiM block (adaLN MHA + adaLN MLP) Trainium2 Bass kernel.

Data-parallel over batch: B=8, one batch element per NeuronCore, weights
replicated, no collectives. Feature-on-partition layout throughout (host
pre-transposes x and the weight matrices; kernel computes out.T, host
transposes back).

Mixed precision: the large matmuls (qkv, attn@v, softmax denominator,
out_proj, both MLP matmuls) run in fp8e4m3 with DoubleRow pairing (two
128-deep k-tiles per instruction); attention scores and LayerNorm
statistics stay float32r. adaLN modulation weights are fp8 and the
modulation matmul uses silu(c) as the 1-column moving operand so its PE
cost is negligible; those chunks are interleaved into attention where the
PE has slack (attention is Act/exp bound). x stays resident in SBUF for
both residual adds. Attention is software-pipelined: softmax-denominator
and attn@v matmuls lag two (head, q-half) units behind scores/exp so the
PE never head-blocks on the Act engine.

Self-contained: hardcodes all shapes; no sibling imports.
"""
import sys

sys.path.insert(0, "/opt/trn_rl_repo")

import numpy as np
import ml_dtypes

import concourse.bass as bass
import concourse.tile as tile
import concourse.mybir as mybir
from concourse import bacc
from concourse.bass_utils import run_bass_kernel_spmd
from concourse.masks import make_identity

D = 1024
N = 1024          # tokens per core
H = 8             # heads
DH = 128
DFF = 4096
KT = D // 128     # feature k-tiles
NT = N // 128     # token tiles
FT = DFF // 128   # mlp f-tiles
EPS = 1e-6
F32 = mybir.dt.float32
F32R = mybir.dt.float32r
F8 = mybir.dt.float8e4
AF = mybir.ActivationFunctionType
ALU = mybir.AluOpType
PM = mybir.MatmulPerfMode

# rows tile: rows 0..10 transposed into smalls columns 0..10 (row 0 = c,
# silu'd in place first -- Act needs partition-0 alignment). rows_d rows
# 11,12 spare; 13 = v bias; 14 = out_proj bias; 15 = b2 (partition-0 tiles).
ROWS_T = 12       # row 11 is padding (PE transpose wants even row count)
NROWS = 16
C_CSIL, C_IPBQ, C_IPBK, C_MG, C_MB, C_FG, C_FB = 0, 1, 2, 3, 4, 5, 6
C_B1 = 7          # 7..10; smalls col 11 = transpose pad
C_SH1, C_SC1, C_SH2, C_SC2, C_G1, C_G2 = 12, 13, 14, 15, 16, 17
C_A1, C_C1, C_A2, C_C2 = 18, 19, 20, 21
NSMALL = 22
# ada chunk order (512-col chunks; 2 per modulation vector):
# m_sh, m_sc, f_sh, f_sc, m_g, f_g
VEC_COL = [C_SH1, C_SC1, C_SH2, C_SC2, C_G1, C_G2]


def f32(ap):
    return ap.bitcast(F32)


PHASES = []


def _mark(nc, label):
    n = nc.get_next_instruction_name()  # consumes one id
    PHASES.append((int(n.split("-")[1]), label))


def _build():
    nc = bacc.Bacc("TRN2")

    xT_d = nc.dram_tensor("xT", [D, N], F32R, kind="ExternalInput")
    rows_d = nc.dram_tensor("rows", [NROWS, D], F32R, kind="ExternalInput")
    ada_d = nc.dram_tensor("ada8", [128, KT, 6 * D], F8, kind="ExternalInput")
    adab_d = nc.dram_tensor("adab", [128, 48], F32, kind="ExternalInput")
    ipqk_d = nc.dram_tensor("ipqk8", [128, H, KT * 256], F8, kind="ExternalInput")
    ipv_d = nc.dram_tensor("ipv8", [128, KT, D], F8, kind="ExternalInput")
    opw_d = nc.dram_tensor("opw8", [128, KT, D], F8, kind="ExternalInput")
    w1_d = nc.dram_tensor("w18", [128, KT, DFF], F8, kind="ExternalInput")
    w2_d = nc.dram_tensor("w28", [128, FT, D], F8, kind="ExternalInput")
    outT = nc.dram_tensor("outT", [D, N], F32, kind="ExternalOutput")

    xT_r = xT_d.rearrange("(kt p) n -> p kt n", p=128)
    inv_sqrt_dh = float(1.0 / np.sqrt(DH))

    with tile.TileContext(nc) as tc, (
        tc.tile_pool(name="persist", bufs=1)
    ) as persist, (
        tc.tile_pool(name="psA", bufs=4, space="PSUM")
    ) as psA, (
        tc.tile_pool(name="psB", bufs=2, space="PSUM")
    ) as psB, (
        tc.tile_pool(name="stream", bufs=1)
    ) as stream:

        # ---------------- persistent tiles -------------------------------
        xT = persist.tile([128, KT, N], F32R, name="xT")        # 32KB/part
        hT = persist.tile([128, KT, N], F8, name="hT")          # 8KB (h1/h2)
        w1 = persist.tile([128, KT, DFF], F8, name="w1")        # 32KB
        w2 = persist.tile([128, FT, D], F8, name="w2")          # 32KB
        smalls = persist.tile([128, KT, NSMALL], F32R, name="smalls")
        adab = persist.tile([128, 48], F32, name="adab")
        csil8 = persist.tile([128, KT, 16], F8, name="csil8")

        ident_r = persist.tile([128, 128], F32R)
        ones_full = persist.tile([128, 128], F32R)
        ones_r = persist.tile([1, 512], F32R)
        negones_r = persist.tile([1, 128], F32R)
        ones8 = persist.tile([128, 2, 128], F8, name="ones8")
        eps_t = persist.tile([128, 1], F32)
        nc.vector.memset(eps_t[:], EPS)
        with tc.tile_pool(name="stage", bufs=1) as stg:
            ident = stg.tile([128, 128], F32)
            make_identity(nc, ident[:])
            nc.vector.tensor_copy(ident_r[:], ident[:])
            onesf = stg.tile([128, 128], F32)
            nc.vector.memset(onesf[:], 1.0)
            nc.vector.tensor_copy(ones_full[:], onesf[:])
            nc.vector.tensor_copy(ones8[:, 0, :], onesf[:])
            nc.vector.tensor_copy(ones8[:, 1, :], onesf[:])
            onesw = stg.tile([1, 512], F32)
            nc.vector.memset(onesw[:], 1.0)
            nc.vector.tensor_copy(ones_r[:], onesw[:])
            negf = stg.tile([1, 128], F32)
            nc.vector.memset(negf[:], -1.0)
            nc.vector.tensor_copy(negones_r[:], negf[:])

        # ---------------- DMAs (ordered by need) --------------------------
        _mark(nc, "setup")
        rows = stream.tile([ROWS_T, D], F32R, tag="rows", bufs=1, name="rows")
        nc.sync.dma_start(rows[:], rows_d[0:ROWS_T, :])
        nc.sync.dma_start(adab[:], adab_d[:])

        ada_tiles = {}

        def ada_chunk_dma(g):
            aw = stream.tile([128, KT, 2048], F8, tag="adaw", bufs=1, name="aw")
            nc.sync.dma_start(aw[:], ada_d[:, :, g * 2048 : (g + 1) * 2048])
            ada_tiles[g] = aw

        nc.sync.dma_start(xT[:, :, 0:512], xT_r[:, :, 0:512])
        ada_chunk_dma(0)       # m_sh | m_sc
        nc.sync.dma_start(xT[:, :, 512:1024], xT_r[:, :, 512:1024])

        # silu(c) (row 0 of rows holds raw c). Act output cannot be
        # fp32r-certified: silu goes to an F32 tmp and a DVE copy rounds it
        # back into the f32r rows tile.
        hp_ctx = tc.high_priority()
        hp_ctx.__enter__()
        csil_tmp = stream.tile([1, D], F32, tag="csiltmp", bufs=1,
                               name="csil_tmp")
        nc.scalar.activation(csil_tmp[:], f32(rows[0:1, :]), AF.Silu)
        nc.vector.tensor_copy(rows[0:1, :], csil_tmp[:])

        def pe_transpose(dst_ap, src_ap, nr):
            tp = psA.tile([128, 512], F32, tag="psA", name="tp")
            nc.tensor.matmul(
                tp[:, :nr].bitcast(F32R), src_ap, ident_r[:nr, :nr],
                is_transpose=True, start=True, stop=True,
            )
            nc.vector.tensor_copy(dst_ap, tp[:, :nr])

        for kt in range(KT):
            pe_transpose(
                smalls[:, kt, 0:ROWS_T], rows[0:ROWS_T, kt * 128 : (kt + 1) * 128],
                ROWS_T,
            )
        nc.vector.tensor_copy(
            csil8[:, :, 0:1], f32(smalls[:, :, C_CSIL : C_CSIL + 1])
        )

        # ---------------- adaLN modulation (fp8 DR, c-moving) -------------
        def mod_vec(vec):
            # modulation vector vec (8 f-tiles of 128) from mega-chunk vec//2
            aw = ada_tiles[vec // 2]
            ofs = (vec % 2) * 1024
            pm = psB.tile([128, 8, 1], F32, tag="psB", name="pm")
            for j in range(8):
                for i in range(KT // 2):
                    nc.tensor.matmul(
                        pm[:, j, :],
                        aw[:, 2 * i : 2 * i + 2, ofs + j * 128 : ofs + (j + 1) * 128],
                        csil8[:, 2 * i : 2 * i + 2, 0:1],
                        start=(i == 0), stop=(i == KT // 2 - 1),
                        perf_mode=PM.DoubleRow,
                    )
            nc.vector.tensor_tensor(
                smalls[:, :, VEC_COL[vec] : VEC_COL[vec] + 1],
                pm[:].bitcast(F32R),
                adab[:, vec * 8 : vec * 8 + 8].bitcast(F32R),
                ALU.add,
            )

        def derive(sc, sh, g_, b_, ca, cc):
            u = stream.tile([128, KT, 1], F32, tag="uderiv", bufs=2, name="u")
            nc.vector.tensor_scalar_add(u[:], f32(smalls[:, :, sc : sc + 1]), 1.0)
            nc.vector.tensor_tensor(
                smalls[:, :, ca : ca + 1], u[:], smalls[:, :, g_ : g_ + 1],
                ALU.mult,
            )
            nc.vector.tensor_tensor(
                smalls[:, :, cc : cc + 1], u[:], smalls[:, :, b_ : b_ + 1],
                ALU.mult,
            )
            nc.vector.tensor_tensor(
                smalls[:, :, cc : cc + 1], smalls[:, :, cc : cc + 1],
                smalls[:, :, sh : sh + 1], ALU.add,
            )

        mod_vec(0)
        mod_vec(1)
        derive(C_SC1, C_SH1, C_MG, C_MB, C_A1, C_C1)
        hp_ctx.__exit__(None, None, None)

        # ---------------- layernorm helpers ------------------------------
        # stats: s1 = col-sums, s2 = col-sums of squares (PE matmuls; squares
        # split DVE/Pool/Act), fused chain -> rstd (bcast tile), mu_row.
        # tiles: mean subtraction ON PE (xc = I.T@x + (-1)^T@mu_row in PSUM),
        # then t1 = xc*rstd (DVE/Pool), h = t1*A + C (Act, fp8 out).
        def ln_stats(srcT, chh, pool, sq_offload=False):
            sl = slice(chh * 512, (chh + 1) * 512)
            s1 = psB.tile([128, 512], F32, tag="psB", name="s1")
            s2 = psB.tile([128, 512], F32, tag="psB", name="s2")
            for kt in range(KT):
                nc.tensor.matmul(
                    s1[:], ones_full[:], srcT[:, kt, sl],
                    start=(kt == 0), stop=(kt == KT - 1),
                )
            for kt in range(KT):
                xsq = pool.tile([128, 512], F32R, tag="xsq", bufs=3, name="xsq")
                if (kt % 4 == 3) or sq_offload:
                    nc.gpsimd.tensor_tensor(
                        xsq[:], f32(srcT[:, kt, sl]), f32(srcT[:, kt, sl]),
                        ALU.mult,
                    )
                else:
                    nc.vector.tensor_tensor(
                        xsq[:], f32(srcT[:, kt, sl]), f32(srcT[:, kt, sl]),
                        ALU.mult,
                    )
                nc.tensor.matmul(
                    s2[:], ones_full[:], xsq[:],
                    start=(kt == 0), stop=(kt == KT - 1),
                )
            mu = pool.tile([128, 512], F32R, tag="mu", bufs=2, name="mu")
            t = pool.tile([128, 512], F32, tag="t", bufs=2, name="t")
            sd = pool.tile([128, 512], F32, tag="sd", bufs=2, name="sd")
            rstd = pool.tile([128, 512], F32, tag="rstd", bufs=2, name="rstd")
            nc.vector.tensor_scalar_mul(mu[:], s1[:], 1.0 / D)
            nc.vector.tensor_tensor(t[:], f32(mu[:]), s1[:], ALU.mult)
            nc.vector.tensor_tensor(sd[:], s2[:], t[:], ALU.subtract)
            nc.scalar.activation(
                sd[:], sd[:], AF.Sqrt, bias=eps_t[:], scale=1.0 / D
            )
            nc.vector.reciprocal(rstd[:], sd[:])
            return (sl, rstd, mu[0:1, :])

        def ln_tiles(srcT, dstT, st, ca, cc, pool, affine_dve=False):
            sl, rstd, mu_row = st
            for kt in range(KT):
                xc = psA.tile([128, 512], F32, tag="psA", name="xc")
                nc.tensor.matmul(
                    xc[:], ident_r[:], srcT[:, kt, sl], start=True, stop=False,
                )
                nc.tensor.matmul(
                    xc[:], negones_r[:], mu_row[:], start=False, stop=True,
                )
                t1 = pool.tile([128, 512], F32, tag="lnt", bufs=4, name="t1")
                nc.vector.tensor_tensor(t1[:], xc[:], rstd[:], ALU.mult)
                if affine_dve:
                    nc.vector.tensor_scalar(
                        dstT[:, kt, sl], t1[:],
                        f32(smalls[:, kt, ca : ca + 1]),
                        f32(smalls[:, kt, cc : cc + 1]),
                        ALU.mult, ALU.add,
                    )
                else:
                    nc.scalar.activation(
                        dstT[:, kt, sl], t1[:], AF.Identity,
                        scale=f32(smalls[:, kt, ca : ca + 1]),
                        bias=f32(smalls[:, kt, cc : cc + 1]),
                    )

        # ---------------- LN1 + v + attention -----------------------------
        with tc.tile_pool(name="pv", bufs=1) as pv:
            v8 = pv.tile([128, NT, D], F8, tag="v8", name="v8")
            oT = pv.tile([128, H, N], F8, tag="oT", name="oT")
            ipv = pv.tile([128, KT, D], F8, tag="ipv", name="ipv")
            opw = pv.tile([128, KT, D], F8, tag="opw", name="opw")
            vbias = pv.tile([1, D], F32R, tag="vbias", name="vbias")
            nc.sync.dma_start(ipv[:], ipv_d[:])
            nc.sync.dma_start(vbias[:], rows_d[13:14, :])

            def emit_v(nts, chhs=(0, 1)):
                for nt in nts:
                    for chh in chhs:
                        sl = slice(chh * 512, (chh + 1) * 512)
                        vp = psA.tile([128, 512], F32, tag="psA", name="vp")
                        for i in range(KT // 2):
                            nc.tensor.matmul(
                                vp[:],
                                hT[:, 2 * i : 2 * i + 2,
                                   nt * 128 : (nt + 1) * 128],
                                ipv[:, 2 * i : 2 * i + 2, sl],
                                start=(i == 0), stop=False,
                                perf_mode=PM.DoubleRow,
                            )
                        nc.tensor.matmul(
                            vp[:], ones_r[:, 0:128], vbias[:, sl],
                            start=False, stop=True,
                        )
                        nc.vector.tensor_copy(v8[:, nt, sl], vp[:])

            _mark(nc, "LN1")
            with tc.tile_pool(name="pln1", bufs=1) as pln1:
                st0 = ln_stats(xT, 0, pln1)
                ln_tiles(xT, hT, st0, C_A1, C_C1, pln1)
                st1 = ln_stats(xT, 1, pln1)
                emit_v(range(0, 4), chhs=(0,))
                ln_tiles(xT, hT, st1, C_A1, C_C1, pln1)
                emit_v(range(4, 8), chhs=(0,))

            _mark(nc, "attn")
            # ---------------- attention (software-pipelined) --------------
            with tc.tile_pool(name="pattn", bufs=1) as pa:
                qkw_tiles = []
                for h in range(H):
                    qkw = pa.tile([128, KT, 256], F8, tag="qkw", bufs=2,
                                  name="qkw")
                    nc.sync.dma_start(qkw[:], ipqk_d[:, h, :])
                    qkw_tiles.append(qkw)
                    if h == 1:
                        ada_chunk_dma(1)
                ada_chunk_dma(2)
                nc.sync.dma_start(opw[:], opw_d[:])
                nc.sync.dma_start(w1[:], w1_d[:])
                nc.sync.dma_start(w2[:], w2_d[:])

                qk_sb = {}

                def emit_qk(h):
                    qkw = qkw_tiles[h]
                    qT = pa.tile([128, N], F32R, tag="qT", bufs=2, name="qT")
                    kTt = pa.tile([128, N], F32R, tag="kTt", bufs=2, name="kTt")
                    for chh in range(2):
                        sl = slice(chh * 512, (chh + 1) * 512)
                        for dst, wofs, bcol in (
                            (qT, 0, C_IPBQ), (kTt, 128, C_IPBK)
                        ):
                            pp = psA.tile([128, 512], F32, tag="psA", name="pp")
                            for i in range(KT // 2):
                                nc.tensor.matmul(
                                    pp[:],
                                    qkw[:, 2 * i : 2 * i + 2, wofs : wofs + 128],
                                    hT[:, 2 * i : 2 * i + 2, sl],
                                    start=(i == 0), stop=(i == KT // 2 - 1),
                                    perf_mode=PM.DoubleRow,
                                )
                            nc.vector.tensor_scalar(
                                dst[:, sl], pp[:],
                                f32(smalls[:, h, bcol : bcol + 1]), None, ALU.add,
                            )
                    qk_sb[h] = (qT, kTt)

                def emit_scores_exp(h, qh):
                    qT, kTt = qk_sb[h]
                    qsl = slice(qh * 512, (qh + 1) * 512)
                    expT = pa.tile(
                        [128, KT, 512], F8, tag="expT", bufs=3, name="expT"
                    )
                    for i in range(KT // 2):
                        sp = psB.tile([128, 2, 512], F32, tag="psB", name="sp")
                        for j in range(2):
                            kt = 2 * i + j
                            nc.tensor.matmul(
                                sp[:, j, :],
                                kTt[:, kt * 128 : (kt + 1) * 128],
                                qT[:, qsl], start=True, stop=True,
                            )
                        nc.scalar.activation(
                            expT[:, 2 * i : 2 * i + 2, :], sp[:], AF.Exp,
                            scale=inv_sqrt_dh,
                        )
                    return expT

                def emit_av(h, qh, expT):
                    qsl = slice(qh * 512, (qh + 1) * 512)
                    lb = psA.tile([128, 512], F32, tag="psA", name="lb")
                    for i in range(KT // 2):
                        nc.tensor.matmul(
                            lb[:], ones8[:], expT[:, 2 * i : 2 * i + 2, :],
                            start=(i == 0), stop=(i == KT // 2 - 1),
                            perf_mode=PM.DoubleRow,
                        )
                    linv = pa.tile([128, 512], F32, tag="linv", bufs=2,
                                   name="linv")
                    nc.vector.reciprocal(linv[:], lb[:])
                    op = psA.tile([128, 512], F32, tag="psA", name="op")
                    for i in range(KT // 2):
                        nc.tensor.matmul(
                            op[:],
                            v8[:, 2 * i : 2 * i + 2, h * 128 : (h + 1) * 128],
                            expT[:, 2 * i : 2 * i + 2, :],
                            start=(i == 0), stop=(i == KT // 2 - 1),
                            perf_mode=PM.DoubleRow,
                        )
                    nc.vector.tensor_tensor(
                        oT[:, h, qsl], op[:], linv[:], ALU.mult
                    )

                # interleave deferred v (heads 4-7 cols) + tiny mod matmuls
                extra = [lambda: emit_v(range(0, 4), chhs=(1,)),
                         lambda: emit_v(range(4, 8), chhs=(1,)),
                         lambda: mod_vec(2), lambda: mod_vec(3),
                         lambda: derive(C_SC2, C_SH2, C_FG, C_FB, C_A2, C_C2),
                         lambda: mod_vec(4), lambda: mod_vec(5)]

                units = [(h, qh) for h in range(H) for qh in range(2)]
                LAG = 2
                pending = []
                for ui, (h, qh) in enumerate(units):
                    if qh == 0:
                        emit_qk(h)
                    pending.append((h, qh, emit_scores_exp(h, qh)))
                    if ui >= 1 and extra:
                        extra.pop(0)()
                    if len(pending) > LAG:
                        ph, pq, pe_ = pending.pop(0)
                        emit_av(ph, pq, pe_)
                while extra:
                    extra.pop(0)()
                for ph, pq, pe_ in pending:
                    emit_av(ph, pq, pe_)

            _mark(nc, "outproj")
            # ---------------- out_proj + residual 1 (chh-major) -----------
            with tc.tile_pool(name="pwo", bufs=1) as pwo:
                opbrow = pwo.tile([1, D], F32R, tag="opbrow", name="opbrow")
                nc.sync.dma_start(opbrow[:], rows_d[14:15, :])
                for chh in range(2):
                    sl = slice(chh * 512, (chh + 1) * 512)
                    for dt_ in range(KT):
                        pp2 = psA.tile([128, 512], F32, tag="psA", name="pp2")
                        for i in range(KT // 2):
                            nc.tensor.matmul(
                                pp2[:],
                                opw[:, 2 * i : 2 * i + 2,
                                    dt_ * 128 : (dt_ + 1) * 128],
                                oT[:, 2 * i : 2 * i + 2, sl],
                                start=(i == 0), stop=False,
                                perf_mode=PM.DoubleRow,
                            )
                        nc.tensor.matmul(
                            pp2[:], opbrow[:, dt_ * 128 : (dt_ + 1) * 128],
                            ones_r[:], start=False, stop=True,
                        )
                        nc.vector.scalar_tensor_tensor(
                            xT[:, dt_, sl], pp2[:],
                            f32(smalls[:, dt_, C_G1 : C_G1 + 1]),
                            f32(xT[:, dt_, sl]), ALU.mult, ALU.add,
                        )

        _mark(nc, "LN2MLP")
        # ---------------- LN2 + MLP + residual 2 (interleaved) ------------
        with tc.tile_pool(name="pln2", bufs=1) as pln2, (
            tc.tile_pool(name="pmlp", bufs=1)
        ) as pm_:
            b2row = pm_.tile([1, D], F32R, tag="b2row", name="b2row")
            nc.sync.dma_start(b2row[:], rows_d[15:16, :])
            gts = {}

            def emit_mlp1(hh):
                tsl = slice(hh * 512, (hh + 1) * 512)
                gT = pm_.tile([128, FT, 512], F8, tag="gT", bufs=2, name="gT")
                for ft in range(FT):
                    gp = psA.tile([128, 512], F32, tag="psA", name="gp")
                    for i in range(KT // 2):
                        nc.tensor.matmul(
                            gp[:],
                            w1[:, 2 * i : 2 * i + 2, ft * 128 : (ft + 1) * 128],
                            hT[:, 2 * i : 2 * i + 2, tsl],
                            start=(i == 0), stop=(i == KT // 2 - 1),
                            perf_mode=PM.DoubleRow,
                        )
                    nc.scalar.activation(
                        gT[:, ft, :], gp[:], AF.Gelu,
                        bias=f32(
                            smalls[:, ft % 8, C_B1 + ft // 8 : C_B1 + ft // 8 + 1]
                        ),
                    )
                gts[hh] = gT

            def emit_mlp2(hh):
                tsl = slice(hh * 512, (hh + 1) * 512)
                gT = gts[hh]
                for dt_ in range(KT):
                    yp = psA.tile([128, 512], F32, tag="psA", name="yp")
                    for i in range(FT // 2):
                        nc.tensor.matmul(
                            yp[:],
                            w2[:, 2 * i : 2 * i + 2, dt_ * 128 : (dt_ + 1) * 128],
                            gT[:, 2 * i : 2 * i + 2, :],
                            start=(i == 0), stop=False,
                            perf_mode=PM.DoubleRow,
                        )
                    nc.tensor.matmul(
                        yp[:], b2row[:, dt_ * 128 : (dt_ + 1) * 128],
                        ones_r[:], start=False, stop=True,
                    )
                    nc.vector.scalar_tensor_tensor(
                        xT[:, dt_, tsl], yp[:],
                        f32(smalls[:, dt_, C_G2 : C_G2 + 1]),
                        f32(xT[:, dt_, tsl]), ALU.mult, ALU.add,
                    )
                    nc.sync.dma_start(
                        outT[dt_ * 128 : (dt_ + 1) * 128, tsl],
                        f32(xT[:, dt_, tsl]),
                    )

            st0 = ln_stats(xT, 0, pln2)
            st1 = ln_stats(xT, 1, pln2)
            ln_tiles(xT, hT, st0, C_A2, C_C2, pln2)
            emit_mlp1(0)
            ln_tiles(xT, hT, st1, C_A2, C_C2, pln2, affine_dve=True)
            emit_mlp2(0)
            emit_mlp1(1)
            emit_mlp2(1)

    nc.compile()
    return nc


_NC_CACHE = None


def _get_nc():
    global _NC_CACHE
    if _NC_CACHE is None:
        _NC_CACHE = _build()
    return _NC_CACHE


def kernel(**inputs):
    B = 8
    f = lambda a: np.ascontiguousarray(np.asarray(a), dtype=np.float32)
    F8NP = ml_dtypes.float8_e4m3fn

    def rearr(w, p=128):
        # [(kt p), f] -> [p, kt, f]
        kt = w.shape[0] // p
        return np.ascontiguousarray(w.reshape(kt, p, -1).transpose(1, 0, 2))

    ipb = f(inputs["in_proj_b"]).reshape(3, D)  # q,k,v rows
    b1r = f(inputs["b1"]).reshape(4, D)
    rows = np.concatenate(
        [
            np.zeros((1, D), np.float32),  # c placeholder (per core, row 0)
            ipb[0:1], ipb[1:2],  # q, k biases
            f(inputs["m_norm_g"]).reshape(1, D),
            f(inputs["m_norm_b"]).reshape(1, D),
            f(inputs["f_norm_g"]).reshape(1, D),
            f(inputs["f_norm_b"]).reshape(1, D),
            b1r,                                    # rows 7..10
            np.zeros((2, D), np.float32),           # spare rows 11,12
            ipb[2:3],                               # v bias row 13
            f(inputs["out_proj_b"]).reshape(1, D),  # row 14
            f(inputs["b2"]).reshape(1, D),          # row 15
        ]
    )
    maw = f(inputs["m_ada_w"])
    faw = f(inputs["f_ada_w"])
    # chunk order: m_sh, m_sc, f_sh, f_sc, m_g, f_g
    ada = np.concatenate(
        [maw[:, 0:D], maw[:, D : 2 * D], faw[:, 0:D], faw[:, D : 2 * D],
         maw[:, 2 * D : 3 * D], faw[:, 2 * D : 3 * D]],
        axis=1,
    )
    ada8 = rearr(ada).astype(F8NP)
    mab = f(inputs["m_ada_b"]).reshape(3 * D)
    fab = f(inputs["f_ada_b"]).reshape(3 * D)
    adab_cat = np.concatenate(
        [mab[0:D], mab[D : 2 * D], fab[0:D], fab[D : 2 * D],
         mab[2 * D : 3 * D], fab[2 * D : 3 * D]]
    )
    # bias for vec v, f-tile j, partition p = adab_cat[v*1024 + j*128 + p]
    adab = np.ascontiguousarray(adab_cat.reshape(48, 128).T)  # [128, 48]

    ipwT = f(inputs["in_proj_w"]).T  # [D_in, 3D_out] cols: q|k|v
    # interleave q/k per head: cols h*256..h*256+128 = q_h, +128..256 = k_h
    qk = np.empty((D, 2 * D), np.float32)
    for h in range(H):
        qk[:, h * 256 : h * 256 + 128] = ipwT[:, h * 128 : (h + 1) * 128]
        qk[:, h * 256 + 128 : (h + 1) * 256] = (
            ipwT[:, D + h * 128 : D + (h + 1) * 128]
        )
    opwT = f(inputs["out_proj_w"]).T

    shared = {
        "ada8": ada8,
        "adab": adab,
        "ipqk8": np.ascontiguousarray(
            rearr(qk).reshape(128, KT, H, 256).transpose(0, 2, 1, 3)
            .reshape(128, H, KT * 256)
        ).astype(F8NP),
        "ipv8": rearr(np.ascontiguousarray(ipwT[:, 2 * D :])).astype(F8NP),
        "opw8": rearr(opwT).astype(F8NP),
        "w18": rearr(f(inputs["w1"])).astype(F8NP),
        "w28": rearr(f(inputs["w2"])).astype(F8NP),
    }
    x = f(inputs["x"])
    c = f(inputs["c"])
    in_maps = []
    for b in range(B):
        rb = rows.copy()
        rb[0] = c[b]
        in_maps.append(
            {"xT": np.ascontiguousarray(x[b].T), **shared, "rows": rb}
        )
    nc = _get_nc()
    br = run_bass_kernel_spmd(nc, in_maps, core_ids=list(range(B)))
    o = np.stack([r["outT"] for r in br.results])  # [B, D, N]
    return np.ascontiguousarray(o.transpose(0, 2, 1)).astype(np.float32)
